# revision 27
# baseline (speedup 1.0000x reference)
"""GAT layer (single head) on 8 Trainium2 NeuronCores.

Strategy: destination-sharded edge parallelism.
  - Nodes padded to NPAD = 8*NB*128; core k owns NB blocks of 128 nodes.
  - Host sorts edges by (dst-core, src-chunk, dst-block) and pads each
    (block, chunk) run to whole tiles of 128 edges (capacity C_bc tiles,
    the max over all runs).
  - Device, per core:
      phase 1: zaug[n] = [z(n) | el(n) | er(n) | 1] for ALL nodes
               (z = h@W.T, el = z@a_l, er = z@a_r) via one fused matmul
               with WAUG = [W.T | wl | wr]; rows stored to DRAM tables
               (256B rows for the gather engine).
      phase 2: per edge tile of 128: dma_gather zaug[src] and er[dst]
               (core-local table, int16-safe); ex = exp(lrelu(el+er));
               one-hot-times-ex matrix via a single two-op tensor_scalar;
               Y[b] += [z|..|1].T @ ohx accumulated in PSUM per node
               block (numerator cols 0:32, denominator col 34);
               out = num / max(denom, eps).
    Softmax max-subtraction is dropped: |e| stays small for this model,
    so exp() is well-conditioned and the softmax ratio is unchanged.

  DRAM node tables use a tiled layout: node n lives at row
  (n % 128) * (NPAD/128) + n // 128, so phase 1 writes them with large
  contiguous per-partition DMA runs; the host bakes this mapping (and
  the 4-way int16 chunking of the z table) into the gather indices.
"""

import sys

sys.path.insert(0, "/opt/trn_rl_repo")

import numpy as np

import concourse.bacc as bacc
import concourse.bass as bass
import concourse.tile as tile
from concourse import mybir
from concourse.bass_utils import run_bass_kernel_spmd
from concourse.masks import make_identity

F32 = mybir.dt.float32
I16 = mybir.dt.int16

N_NODES = 100000
IN_FEATS = 128
OUT_FEATS = 32
NEG_SLOPE = 0.2
N_CORES = 8
BLK = 128
NB = 98  # blocks per core (full problem)
EL = 64  # table row: 64 f32 = 256B (dma_gather granularity)
NQ = 4  # int16 chunks of the z table
ZG = 512  # nodes per z-phase group
BGS = 14  # blocks per phase-2 group

C_EL = 32  # el column in zaug row
C_ER = 33  # er column
C_ONE = 34  # constant-one column

_cache = {}


def _build(C, nb=NB, bgs=BGS, dbg=False):
    """C = tiles of 128 edges per (block, chunk) run."""
    assert nb % bgs == 0, (nb, bgs)
    core_nodes = nb * BLK
    npad = N_CORES * core_nodes
    ncols = npad // BLK
    chunk_rows = (BLK // NQ) * ncols  # z-table rows per int16 chunk
    assert chunk_rows < 32768 and core_nodes < 32768
    nzg = npad // ZG
    sub = ZG // BLK
    T = NQ * nb * C  # tile columns per core
    NW = T * BLK // 16  # wrapped-index columns

    nc = bacc.Bacc("TRN2", target_bir_lowering=False, debug=False,
                   num_devices=N_CORES)

    hT = nc.dram_tensor("hT", [IN_FEATS, npad], F32, kind="ExternalInput")
    Wt = nc.dram_tensor("Wt", [OUT_FEATS, IN_FEATS], F32, kind="ExternalInput")
    av = nc.dram_tensor("av", [2 * OUT_FEATS, 1], F32, kind="ExternalInput")
    srcw = nc.dram_tensor("srcw", [BLK, NW], I16, kind="ExternalInput")
    erw = nc.dram_tensor("erw", [BLK, NW], I16, kind="ExternalInput")
    dstloc = nc.dram_tensor("dstloc", [BLK, T], F32, kind="ExternalInput")
    out = nc.dram_tensor("out", [core_nodes, OUT_FEATS], F32,
                         kind="ExternalOutput")

    zaug = nc.dram_tensor("zaug", [npad, EL], F32)
    ertab = nc.dram_tensor("ertab", [core_nodes, EL], F32)

    if dbg:
        NCOL0 = bgs * C
        zg_d = nc.dram_tensor("zg_d", [NQ, BLK, NCOL0, EL], F32,
                              kind="ExternalOutput")
        erg_d = nc.dram_tensor("erg_d", [NQ, BLK, NCOL0, EL], F32,
                               kind="ExternalOutput")
        ex_d = nc.dram_tensor("ex_d", [NQ, BLK, NCOL0], F32,
                              kind="ExternalOutput")
        y_d = nc.dram_tensor("y_d", [bgs, 64, BLK], F32,
                             kind="ExternalOutput")
        oh_d = nc.dram_tensor("oh_d", [BLK, BLK], F32, kind="ExternalOutput")
        ertab_d = nc.dram_tensor("ertab_d", [core_nodes, 1], F32,
                                 kind="ExternalOutput")

    with tile.TileContext(nc) as tc:
        with tc.tile_pool(name="const", bufs=1) as cpool:
            ident = cpool.tile([128, 128], F32)
            make_identity(nc, ident[:])
            iota = cpool.tile([128, BLK], F32)
            nc.gpsimd.iota(iota[:], pattern=[[1, BLK]], base=0,
                           channel_multiplier=0,
                           allow_small_or_imprecise_dtypes=True)

            # WAUG = [W.T | wl | wr]  (wl = W.T a_l, wr = W.T a_r)
            waug = cpool.tile([IN_FEATS, C_ONE], F32)
            nc.vector.memset(waug[:], 0.0)
            with tc.tile_pool(name="wprep", bufs=1) as wpool, \
                 tc.tile_pool(name="wpsum", bufs=2, space="PSUM") as wps:
                w_sb = wpool.tile([OUT_FEATS, IN_FEATS], F32)
                nc.sync.dma_start(out=w_sb[:], in_=Wt[:])
                al_sb = wpool.tile([OUT_FEATS, 1], F32)
                nc.sync.dma_start(out=al_sb[:], in_=av[0:OUT_FEATS, :])
                ar_sb = wpool.tile([OUT_FEATS, 1], F32)
                nc.sync.dma_start(out=ar_sb[:],
                                  in_=av[OUT_FEATS:2 * OUT_FEATS, :])
                wt_ps = wps.tile([IN_FEATS, OUT_FEATS], F32)
                nc.tensor.transpose(out=wt_ps[:], in_=w_sb[:],
                                    identity=ident[0:OUT_FEATS, 0:OUT_FEATS])
                nc.vector.tensor_copy(out=waug[:, 0:OUT_FEATS], in_=wt_ps[:])
                wl_ps = wps.tile([IN_FEATS, 1], F32)
                nc.tensor.matmul(out=wl_ps[:], lhsT=w_sb[:],
                                 rhs=al_sb[:], start=True, stop=True)
                nc.vector.tensor_copy(out=waug[:, C_EL:C_EL + 1],
                                      in_=wl_ps[:])
                wr_ps = wps.tile([IN_FEATS, 1], F32)
                nc.tensor.matmul(out=wr_ps[:], lhsT=w_sb[:],
                                 rhs=ar_sb[:], start=True, stop=True)
                nc.vector.tensor_copy(out=waug[:, C_ER:C_ER + 1],
                                      in_=wr_ps[:])

            # ---------------- phase 1: build zaug / ertab ----------------
            er_sb = cpool.tile([BLK, ncols], F32)
            zaug_t = zaug.ap().rearrange("(p c) z -> p c z", p=BLK)
            with tc.tile_pool(name="zh", bufs=3) as hpool, \
                 tc.tile_pool(name="zps", bufs=4, space="PSUM") as zps, \
                 tc.tile_pool(name="zrow", bufs=3) as rpool:
                for g in range(nzg):
                    n0 = g * ZG
                    htile = hpool.tile([IN_FEATS, ZG], F32)
                    nc.sync.dma_start(out=htile[:], in_=hT[:, n0:n0 + ZG])
                    zrows = rpool.tile([128, sub, EL], F32)
                    nc.vector.memset(zrows[:, :, C_ONE:C_ONE + 1], 1.0)
                    for s in range(sub):
                        z_ps = zps.tile([128, C_ONE], F32)
                        nc.tensor.matmul(
                            out=z_ps[:],
                            lhsT=htile[:, s * BLK:(s + 1) * BLK],
                            rhs=waug[:], start=True, stop=True)
                        nc.scalar.copy(out=zrows[:, s, 0:C_ONE], in_=z_ps[:])
                        nc.vector.tensor_copy(
                            out=er_sb[:, g * sub + s:g * sub + s + 1],
                            in_=z_ps[:, C_ER:C_ER + 1])
                    nc.sync.dma_start(
                        out=zaug_t[:, g * sub:(g + 1) * sub, :],
                        in_=zrows[:])
                pid = nc.gpsimd.partition_id()
                ertab_t = ertab.ap().rearrange("(p b) e -> p b e", p=BLK)
                nc.gpsimd.dma_start(
                    out=ertab_t[:, :, 0:1],
                    in_=er_sb[:, bass.ts(pid, nb), None])
                if dbg:
                    nc.gpsimd.dma_start(
                        out=ertab_d.ap().rearrange("(p b) e -> p b e",
                                                   p=BLK),
                        in_=er_sb[:, bass.ts(pid, nb), None])

            # ---------------- phase 2: edges ----------------
            with tc.tile_pool(name="ix", bufs=1) as ixpool:
                srcw_sb = ixpool.tile([BLK, NW], I16)
                nc.sync.dma_start(out=srcw_sb[:], in_=srcw[:])
                erw_sb = ixpool.tile([BLK, NW], I16)
                nc.sync.dma_start(out=erw_sb[:], in_=erw[:])
                dl_sb = ixpool.tile([BLK, T], F32)
                nc.sync.dma_start(out=dl_sb[:], in_=dstloc[:])

                with tc.tile_pool(name="zg", bufs=3) as zgpool, \
                     tc.tile_pool(name="erg", bufs=3) as erpool, \
                     tc.tile_pool(name="ex", bufs=4) as expool, \
                     tc.tile_pool(name="oh", bufs=12) as ohpool, \
                     tc.tile_pool(name="acc", bufs=2) as apool, \
                     tc.tile_pool(name="yps", bufs=5, space="PSUM") as ypool, \
                     tc.tile_pool(name="ytp", bufs=3, space="PSUM") as ytpool, \
                     tc.tile_pool(name="fin", bufs=6) as fpool, \
                     tc.tile_pool(name="ost", bufs=2) as opool:
                    NCOL = bgs * C  # tile columns per (group, chunk)
                    NY = C_ONE + 1
                    for bg in range(nb // bgs):
                        acc = apool.tile([NY, bgs, BLK], F32)
                        nc.vector.memset(acc[:], 0.0)
                        for q in range(NQ):
                            colbase = q * nb * C + bg * NCOL
                            nidx = NCOL * BLK
                            w0 = colbase * BLK // 16
                            zg = zgpool.tile([BLK, NCOL, EL], F32)
                            erg = erpool.tile([BLK, NCOL, EL], F32)
                            # SWDGE ring cap: <=1024 indices per call
                            GCH = 8  # tile-columns per call (1024 idxs)
                            for j0 in range(0, NCOL, GCH):
                                j1 = min(j0 + GCH, NCOL)
                                ni = (j1 - j0) * BLK
                                wj = w0 + j0 * BLK // 16
                                nc.gpsimd.dma_gather(
                                    out_ap=zg[:, j0:j1, :],
                                    in_ap=zaug[q * chunk_rows:
                                               (q + 1) * chunk_rows, :],
                                    idxs_ap=srcw_sb[:, wj:wj + ni // 16],
                                    num_idxs=ni, num_idxs_reg=ni,
                                    elem_size=EL)
                                nc.gpsimd.dma_gather(
                                    out_ap=erg[:, j0:j1, :], in_ap=ertab[:],
                                    idxs_ap=erw_sb[:, wj:wj + ni // 16],
                                    num_idxs=ni, num_idxs_reg=ni,
                                    elem_size=EL)
                            ex = expool.tile([BLK, NCOL], F32)
                            sv = expool.tile([BLK, NCOL], F32, tag="sv")
                            nc.vector.tensor_add(out=sv[:],
                                                 in0=zg[:, :, C_EL],
                                                 in1=erg[:, :, 0])
                            # leaky_relu(x) = max(x, 0.2x); the Lrelu ACT
                            # table has a baked-in 0.01 slope, so do it
                            # manually
                            nc.vector.tensor_scalar(
                                ex[:], sv[:], NEG_SLOPE, None,
                                mybir.AluOpType.mult)
                            nc.vector.tensor_tensor(
                                out=ex[:], in0=ex[:], in1=sv[:],
                                op=mybir.AluOpType.max)
                            nc.scalar.activation(
                                out=ex[:], in_=ex[:],
                                func=mybir.ActivationFunctionType.Exp)
                            if dbg and bg == 0:
                                nc.sync.dma_start(out=zg_d[q], in_=zg[:])
                                nc.sync.dma_start(out=erg_d[q], in_=erg[:])
                                nc.sync.dma_start(out=ex_d[q], in_=ex[:])
                            for b in range(bgs):
                                y_ps = ypool.tile([NY, BLK], F32)
                                for t in range(C):
                                    lcol = b * C + t
                                    col = colbase + lcol
                                    oh = ohpool.tile([BLK, BLK], F32)
                                    nc.vector.tensor_scalar(
                                        oh[:], iota[:],
                                        dl_sb[:, col:col + 1],
                                        ex[:, lcol:lcol + 1],
                                        mybir.AluOpType.is_equal,
                                        mybir.AluOpType.mult)
                                    if dbg and bg == 0 and q == 0 and b == 0 \
                                            and t == 0:
                                        nc.sync.dma_start(out=oh_d[:],
                                                          in_=oh[:])
                                    nc.tensor.matmul(
                                        out=y_ps[:],
                                        lhsT=zg[:, lcol, 0:NY],
                                        rhs=oh[:],
                                        start=(t == 0),
                                        stop=(t == C - 1))
                                nc.vector.tensor_add(out=acc[:, b, :],
                                                     in0=acc[:, b, :],
                                                     in1=y_ps[:])
                        ost = opool.tile([BLK, bgs, OUT_FEATS], F32)
                        for b in range(bgs):
                            if dbg and bg == 0:
                                nc.sync.dma_start(out=y_d[b, 0:NY, :],
                                                  in_=acc[:, b, :])
                            yt = ytpool.tile([BLK, NY], F32)
                            nc.tensor.transpose(out=yt[:], in_=acc[:, b, :],
                                                identity=ident[0:NY, 0:NY])
                            den = fpool.tile([BLK, 1], F32)
                            nc.vector.tensor_scalar(
                                den[:], yt[:, C_ONE:C_ONE + 1], 1e-16, None,
                                mybir.AluOpType.max)
                            rden = fpool.tile([BLK, 1], F32)
                            nc.vector.reciprocal(out=rden[:], in_=den[:])
                            nc.vector.tensor_scalar(
                                ost[:, b, :], yt[:, 0:OUT_FEATS], rden[:],
                                None, mybir.AluOpType.mult)
                        n0 = bg * bgs * BLK
                        nc.sync.dma_start(
                            out=out[n0:n0 + bgs * BLK, :].rearrange(
                                "(s p) c -> p s c", p=BLK),
                            in_=ost[:])

    nc.compile()
    return nc


def _prep(h, W, a, src, dst, nb=NB, n_nodes=N_NODES):
    """Host-side sharding / index layout (integer index manipulation and
    zero-padding only - all floating-point math runs on device)."""
    core_nodes = nb * BLK
    npad = N_CORES * core_nodes
    ncols = npad // BLK
    chunk_rows = (BLK // NQ) * ncols

    h = np.asarray(h, dtype=np.float32)
    W = np.ascontiguousarray(np.asarray(W, dtype=np.float32))
    a = np.asarray(a, dtype=np.float32).reshape(-1)
    src = np.asarray(src, dtype=np.int64)
    dst = np.asarray(dst, dtype=np.int64)

    hT = np.zeros((IN_FEATS, npad), dtype=np.float32)
    hT[:, :n_nodes] = h.T
    av = np.ascontiguousarray(a.reshape(-1, 1), dtype=np.float32)

    core = dst // core_nodes
    b_of = (dst % core_nodes) // BLK
    q_of = (src % BLK) // (BLK // NQ)
    grp = (core * NQ + q_of) * nb + b_of
    order = np.argsort(grp, kind="stable")
    gs = grp[order]
    ss = src[order]
    ds = dst[order]

    counts = np.bincount(gs, minlength=N_CORES * NQ * nb)
    C = int(max(1, -(-counts.max() // BLK)))
    T = NQ * nb * C
    NW = T * BLK // 16

    # global slot of each sorted edge
    starts = np.zeros(N_CORES * NQ * nb + 1, dtype=np.int64)
    np.cumsum(counts, out=starts[1:])
    rank = np.arange(len(ss)) - starts[gs]
    # within-core group index: (q * nb + b) for that core
    gloc = gs % (NQ * nb)
    slot = gloc * (C * BLK) + rank  # slot within the core's edge buffer

    src_t = (ss % BLK) * ncols + ss // BLK  # tiled z-table row
    src_i16 = (src_t - q_of[order] * chunk_rows).astype(np.int16)
    er_i16_all = ((ds % BLK) * nb + (ds % core_nodes) // BLK).astype(np.int16)
    dl_all = (ds % core_nodes - b_of[order] * BLK).astype(np.float32)

    srcw = np.zeros((N_CORES, BLK, NW), dtype=np.int16)
    erw = np.zeros((N_CORES, BLK, NW), dtype=np.int16)
    dstloc = np.full((N_CORES, BLK, T), -1.0, dtype=np.float32)
    for k in range(N_CORES):
        m = core[order] == k
        sl = slot[m]
        sflat = np.zeros(T * BLK, dtype=np.int16)
        eflat = np.zeros(T * BLK, dtype=np.int16)
        dflat = np.full(T * BLK, -1.0, dtype=np.float32)
        sflat[sl] = src_i16[m]
        eflat[sl] = er_i16_all[m]
        dflat[sl] = dl_all[m]
        # wrapped-16, replicated over the 8 gpsimd groups
        srcw[k] = np.tile(sflat.reshape(-1, 16).T, (8, 1))
        erw[k] = np.tile(eflat.reshape(-1, 16).T, (8, 1))
        dstloc[k] = dflat.reshape(T, BLK).T
    return hT, W, av, srcw, erw, dstloc, C


def kernel(h, W, a, src, dst):
    hT, Wm, av, srcw, erw, dstloc, C = _prep(h, W, a, src, dst)
    if C not in _cache:
        _cache[C] = _build(C)
    nc = _cache[C]
    in_maps = []
    for k in range(N_CORES):
        in_maps.append({
            "hT": hT,
            "Wt": Wm,
            "av": av,
            "srcw": srcw[k],
            "erw": erw[k],
            "dstloc": dstloc[k],
        })
    res = run_bass_kernel_spmd(nc, in_maps, list(range(N_CORES)))
    outs = [res.results[k]["out"] for k in range(N_CORES)]
    full = np.concatenate(outs, axis=0)[:N_NODES]
    return np.ascontiguousarray(full, dtype=np.float32)
